# revision 25
# baseline (speedup 1.0000x reference)
"""Trainium2 Bass kernel for gated pair-bias attention (AlphaFold-style).

Reference computation per (b=1, n) row:
  q,k,v = proj(input_*) reshaped to [H=8, S=256, C=32]; q /= sqrt(32)
  a = softmax(q@k^T + (mask-1)*1e9 + bias)      # [H, Q, K]
  o = (a@v) * sigmoid(input_q@wg + bg)          # gated
  out = o @ wo + bo                             # [S, 128]

Sharding: dim 1 (N=256 rows) split across 8 cores, 32 rows/core.
Layout strategy (per row, all bf16 matmuls, fp32 PSUM):
  - host pre-transposes x to [c=128, s=256] (pure layout prep)
  - qT,kT computed transposed [d,s]; v,g computed natural [s,d]
  - logits computed transposed e^T[k,q] per head (K=32 row-group packed)
  - softmax without max-subtraction (logits are small); Z via an
    em-column appended to v in the AV matmul (em = exp((mask-1)*1e9))
  - exp(logits)*exp(bias^T) product form; bias^T preloaded once per core
  - AV natural: o[q, (h,33)] with Z in col 32 of each 33-block
  - normalize+gate on DVE, PE-transpose og -> og^T, final matmul natural
"""

import math
import sys

sys.path.insert(0, "/opt/trn_rl_repo")

import numpy as np
import ml_dtypes

BF16 = ml_dtypes.bfloat16

B, N, S, CQ = 1, 256, 256, 128
H, C = 8, 32
NCORES = 8
NPER = N // NCORES  # 32 rows per core


def _build_bass():
    import concourse.bass as bass
    import concourse.bacc as bacc
    import concourse.tile as tile
    from concourse import mybir
    from concourse.masks import make_identity

    dt = mybir.dt
    AF = mybir.ActivationFunctionType
    ALU = mybir.AluOpType

    nc = bacc.Bacc()

    # ---- DRAM parameters (per-core shapes) ----
    x_all = nc.declare_dram_parameter("x_all", [NPER, 3, CQ, S], dt.bfloat16, isOutput=False)
    maskT = nc.declare_dram_parameter("maskT", [CQ, 2 * NPER], dt.float32, isOutput=False)
    biasT = nc.declare_dram_parameter("biasT", [2, CQ, H, S], dt.bfloat16, isOutput=False)
    wq = nc.declare_dram_parameter("wq", [2, 4, CQ, CQ], dt.bfloat16, isOutput=False)
    wk = nc.declare_dram_parameter("wk", [CQ, H * C], dt.bfloat16, isOutput=False)
    wv = nc.declare_dram_parameter("wv", [CQ, H * C], dt.bfloat16, isOutput=False)
    wg = nc.declare_dram_parameter("wg", [CQ, H * C], dt.bfloat16, isOutput=False)
    wo_t = nc.declare_dram_parameter("wo_t", [CQ, 2 * CQ], dt.bfloat16, isOutput=False)
    bg = nc.declare_dram_parameter("bg", [1, H * C], dt.bfloat16, isOutput=False)
    bo = nc.declare_dram_parameter("bo", [1, CQ], dt.float32, isOutput=False)
    out_d = nc.declare_dram_parameter("out", [NPER, S, CQ], dt.float32, isOutput=True)

    with tile.TileContext(nc) as tc:
        with (
            tc.tile_pool(name="const", bufs=1) as const,
            tc.tile_pool(name="xp", bufs=4) as xp,
            tc.tile_pool(name="qk", bufs=2) as qkp,
            tc.tile_pool(name="ep", bufs=8) as ep,
            tc.tile_pool(name="eraw", bufs=3) as erawp,
            tc.tile_pool(name="vap", bufs=4) as vap,
            tc.tile_pool(name="gp", bufs=2) as gpool,
            tc.tile_pool(name="ogp", bufs=4) as ogp,
            tc.tile_pool(name="zp", bufs=4) as zp,
            tc.tile_pool(name="outp", bufs=2) as outp,
            tc.tile_pool(name="psbig", bufs=2, space="PSUM") as psbig,
            tc.tile_pool(name="psqkvg", bufs=2, space="PSUM") as psqkvg,
            tc.tile_pool(name="pspm", bufs=2, space="PSUM") as pspm,
        ):
            # ---------- once-per-core setup ----------
            wq_t = const.tile([CQ, 2, 4, CQ], dt.bfloat16, tag="wq")
            wk_t = const.tile([CQ, H * C], dt.bfloat16, tag="wk")
            wv_t = const.tile([CQ, H * C], dt.bfloat16, tag="wv")
            wg_t = const.tile([CQ, H * C], dt.bfloat16, tag="wg")
            wo_tt = const.tile([CQ, 2 * CQ], dt.bfloat16, tag="wo")
            bg_t = const.tile([1, H * C], dt.bfloat16, tag="bg")
            bo_t = const.tile([CQ, CQ], dt.float32, tag="bo")
            ones_c = const.tile([1, CQ], dt.bfloat16, tag="ones")
            ident = const.tile([CQ, CQ], dt.bfloat16, tag="ident")
            emf = const.tile([CQ, 2 * NPER], dt.float32, tag="emf")
            emb = const.tile([CQ, 2 * NPER], dt.bfloat16, tag="emb")
            ebr = const.tile([CQ, 2, H * S], dt.bfloat16, tag="ebr")
            eb = const.tile([CQ, 2, H * S], dt.bfloat16, tag="eb")
            mt_t = const.tile([CQ, 2 * NPER], dt.float32, tag="mt")

            nc.sync.dma_start(out=wq_t, in_=wq.ap().rearrange("a g p m -> p a g m"))
            nc.sync.dma_start(out=wk_t, in_=wk[:, :])
            nc.sync.dma_start(out=wv_t, in_=wv[:, :])
            nc.sync.dma_start(out=wg_t, in_=wg[:, :])
            nc.sync.dma_start(out=wo_tt, in_=wo_t[:, :])
            nc.sync.dma_start(out=bg_t, in_=bg[:, :])
            nc.sync.dma_start(out=mt_t, in_=maskT[:, :])
            # broadcast bo across 128 partitions
            bo_ap0 = bo[:, :]
            bo_bc_ap = bass.AP(tensor=bo_ap0.tensor, offset=bo_ap0.offset,
                               ap=[[0, CQ], [1, CQ]])
            nc.sync.dma_start(out=bo_t, in_=bo_bc_ap)
            for kc in range(2):
                nc.sync.dma_start(
                    out=ebr[:, kc],
                    in_=biasT[kc].rearrange("p h q -> p (h q)"),
                )
            nc.vector.memset(ones_c, 1.0)
            make_identity(nc, ident)
            # em = exp((mask-1)*1e9) as [k, (kc, n)] per-partition scalars
            nc.vector.tensor_scalar(out=emf, in0=mt_t, scalar1=1.0, scalar2=1.0e9,
                                    op0=ALU.subtract, op1=ALU.mult)
            nc.scalar.activation(emf, emf, AF.Exp)
            nc.vector.tensor_copy(out=emb, in_=emf)
            # expbiasT = exp(bias^T)  [128, kc, (h q)]
            for kc in range(2):
                nc.scalar.activation(eb[:, kc], ebr[:, kc], AF.Exp)

            # ---------- per-row pipeline ----------
            for n in range(NPER):
                # x^T tiles: [c=128, (t, s)] for t in (q, k, v)
                xt = xp.tile([CQ, 3, S], dt.bfloat16, tag="xt")
                nc.sync.dma_start(out=xt, in_=x_all[n].rearrange("t p s -> p t s"))
                xqT, xkT, xvT = xt[:, 0], xt[:, 1], xt[:, 2]

                # projections: k transposed [d, s]; q block-diagonal per head
                qbd = []
                for dc in range(2):
                    ps_qb = psbig.tile([CQ, 4, S], dt.float32, tag="big")
                    for g in range(4):
                        nc.tensor.matmul(ps_qb[:, g],
                                         wq_t[:, dc, g], xqT,
                                         start=True, stop=True)
                    qb = qkp.tile([CQ, 4, S], dt.bfloat16, tag="qbd")
                    nc.vector.tensor_copy(out=qb[:, 0:2], in_=ps_qb[:, 0:2])
                    nc.vector.tensor_copy(out=qb[:, 2:4], in_=ps_qb[:, 2:4])
                    qbd.append(qb)
                ps_k = psqkvg.tile([CQ, 2 * S], dt.float32, tag="psqkvg")
                for dc in range(2):
                    nc.tensor.matmul(ps_k[:, dc * S:(dc + 1) * S],
                                     wk_t[:, dc * CQ:(dc + 1) * CQ], xkT,
                                     start=True, stop=True)
                kT = qkp.tile([CQ, 2 * S], dt.bfloat16, tag="kT")
                nc.scalar.copy(out=kT, in_=ps_k)

                ps_v = psqkvg.tile([CQ, 2 * S], dt.float32, tag="psqkvg")
                ps_g = psqkvg.tile([CQ, 2 * S], dt.float32, tag="psqkvg")
                for sc in range(2):
                    nc.tensor.matmul(ps_v[:, sc * S:(sc + 1) * S],
                                     xvT[:, sc * CQ:(sc + 1) * CQ], wv_t,
                                     start=True, stop=True)
                    nc.tensor.matmul(ps_g[:, sc * S:(sc + 1) * S],
                                     xqT[:, sc * CQ:(sc + 1) * CQ], wg_t,
                                     start=True, stop=False)
                    nc.tensor.matmul(ps_g[:, sc * S:(sc + 1) * S],
                                     ones_c, bg_t, start=False, stop=True)

                # gate g = sigmoid(z) = 0.5*tanh(0.5 z)+0.5, natural [q,(sc,d)]
                g_t = gpool.tile([CQ, 2 * S], dt.bfloat16, tag="g")
                nc.scalar.activation(g_t, ps_g, AF.Tanh, scale=0.5)
                nc.vector.tensor_scalar(out=g_t, in0=g_t, scalar1=0.5, scalar2=0.5,
                                        op0=ALU.mult, op1=ALU.add)

                # v_aug [k-chunk, (h, 33)]: v*em | em
                vas = []
                for kc in range(2):
                    va = vap.tile([CQ, H, 33], dt.bfloat16, tag="va")
                    nc.scalar.activation(
                        va[:, :, 0:32],
                        ps_v[:, kc * S:(kc + 1) * S].rearrange("p (h x) -> p h x", x=32),
                        AF.Copy,
                        scale=emf[:, kc * NPER + n: kc * NPER + n + 1],
                    )
                    nc.vector.tensor_copy(
                        out=va[:, :, 32],
                        in_=emb[:, kc * NPER + n: kc * NPER + n + 1].broadcast_to((CQ, H)),
                    )
                    vas.append(va)

                # logits^T e[k, (g, q)] per (dc, kc): lhsT = full kT chunk,
                # rhs = block-diag q (zeros isolate each head) -> exp -> *expbias
                efin = [[None, None], [None, None]]
                for dc in range(2):
                    for kc in range(2):
                        ps_e = psbig.tile([CQ, 4, S], dt.float32, tag="big")
                        for gp in range(2):
                            nc.tensor.matmul(
                                ps_e[:, 2 * gp:2 * gp + 2, :],
                                kT[:, dc * S + kc * CQ: dc * S + kc * CQ + CQ],
                                qbd[dc][:, 2 * gp:2 * gp + 2, :],
                                start=True, stop=True,
                            )
                        er = erawp.tile([CQ, 4 * S], dt.bfloat16, tag="eraw")
                        nc.scalar.activation(er, ps_e.rearrange("p g s -> p (g s)"), AF.Exp)
                        ef = ep.tile([CQ, 4 * S], dt.bfloat16, tag="efin")
                        nc.vector.tensor_mul(
                            ef, er, eb[:, kc, dc * 4 * S:(dc + 1) * 4 * S])
                        efin[dc][kc] = ef

                # AV + normalize + gate; h/kc outer, qc inner (lhsT reuse)
                ogs = []
                ps_o0 = pspm.tile([CQ, H * 33], dt.float32, tag="pspm")
                ps_o1 = pspm.tile([CQ, H * 33], dt.float32, tag="pspm")
                ps_os = [ps_o0, ps_o1]
                for h in range(H):
                    hg, hh = h // 4, h % 4
                    for kc in range(2):
                        for qc in range(2):
                            nc.tensor.matmul(
                                ps_os[qc][:, h * 33:(h + 1) * 33],
                                efin[hg][kc][:, hh * S + qc * CQ: hh * S + qc * CQ + CQ],
                                vas[kc][:, h, :],
                                start=(kc == 0), stop=(kc == 1),
                            )
                for qc in range(2):
                    ps_o = ps_os[qc]
                    ps_o_r = ps_o.rearrange("p (h x) -> p h x", x=33)
                    zt = zp.tile([CQ, H], dt.float32, tag="zt")
                    rz = zp.tile([CQ, H], dt.float32, tag="rz")
                    nc.vector.tensor_copy(out=zt, in_=ps_o_r[:, :, 32])
                    nc.vector.reciprocal(out=rz, in_=zt)
                    og1 = ogp.tile([CQ, H, 32], dt.bfloat16, tag="og1")
                    nc.vector.tensor_mul(
                        og1, ps_o_r[:, :, 0:32],
                        rz.unsqueeze(2).broadcast_to((CQ, H, 32)))
                    og = ogp.tile([CQ, H * 32], dt.bfloat16, tag="og")
                    nc.vector.tensor_mul(
                        og, og1.rearrange("p h x -> p (h x)"),
                        g_t[:, qc * S:(qc + 1) * S])
                    ogs.append(og)

                # og^T via PE transpose, then final matmul (natural out)
                ogT = []
                for dc in range(2):
                    pt = pspm.tile([CQ, 2 * CQ], dt.bfloat16, tag="pspm")
                    for qc in range(2):
                        nc.tensor.transpose(
                            pt[:, qc * CQ:(qc + 1) * CQ],
                            ogs[qc][:, dc * CQ:(dc + 1) * CQ], ident)
                    ot = ogp.tile([CQ, 2 * CQ], dt.bfloat16, tag="ogT")
                    nc.scalar.copy(out=ot, in_=pt)
                    ogT.append(ot)

                ps_out = pspm.tile([CQ, 2 * CQ], dt.float32, tag="pspm")
                for sc in range(2):
                    for dc in range(2):
                        nc.tensor.matmul(
                            ps_out[:, sc * CQ:(sc + 1) * CQ],
                            ogT[dc][:, sc * CQ:(sc + 1) * CQ],
                            wo_tt[:, dc * CQ:(dc + 1) * CQ],
                            start=(dc == 0), stop=(dc == 1))
                out_sb = outp.tile([CQ, 2, CQ], dt.float32, tag="osb")
                nc.vector.tensor_add(
                    out_sb, ps_out.rearrange("p (sc c) -> p sc c", c=CQ),
                    bo_t.unsqueeze(1).broadcast_to((CQ, 2, CQ)))
                nc.sync.dma_start(
                    out=out_d[n].rearrange("(sc p) c -> p sc c", p=CQ),
                    in_=out_sb)
    if not nc.is_finalized():
        nc.finalize()
    return nc


_NC_CACHE = None


def _get_nc():
    global _NC_CACHE
    if _NC_CACHE is None:
        _NC_CACHE = _build_bass()
    return _NC_CACHE


_LDW_PATCHED = False


def _enable_ldw_opt():
    """Rewrite walrus argv to enable LDWEIGHTS elision for back-to-back
    matmuls sharing a stationary operand (correctness re-checked vs ref)."""
    global _LDW_PATCHED
    if _LDW_PATCHED:
        return
    from concourse import bass_utils as bu

    orig = bu.run_command

    def patched(argv, **kw):
        pass  # ldw-opt=true breaks walrus codegen (visitInstLdweights)
        return orig(argv, **kw)

    bu.run_command = patched
    _LDW_PATCHED = True


def kernel(input_q, input_k, input_v, mask, bias, wq, wk, wv, wg, bg, wo, bo):
    from concourse.bass_utils import run_bass_kernel_spmd

    _enable_ldw_opt()

    nc = _get_nc()

    # ---- host-side input prep (sharding + layout) ----
    wq_s = (wq / math.sqrt(C)).astype(np.float32)   # fold q-scaling into wq
    # block-diagonal zero-padded q weights: wqbd[dc, g, c, m] = wq_s[c, dc*128+m]
    # if m//32 == g else 0  -> per-head logits with base-0 matmul operands
    wqbd = np.zeros((2, 4, CQ, CQ), dtype=np.float32)
    for dc in range(2):
        for g in range(4):
            wqbd[dc, g, :, g * 32:(g + 1) * 32] = wq_s[:, dc * CQ + g * 32: dc * CQ + (g + 1) * 32]
    wqbd = wqbd.astype(BF16)
    wk_b, wv_b, wg_b = wk.astype(BF16), wv.astype(BF16), wg.astype(BF16)
    # wo [256,128] -> [128, (dc,128)]: wo_t[p, dc*128+c] = wo[dc*128+p, c]
    wo_tt = np.ascontiguousarray(
        wo.reshape(2, CQ, CQ).transpose(1, 0, 2).reshape(CQ, 2 * CQ)).astype(BF16)
    bg_b = bg.reshape(1, H * C).astype(BF16)
    bo_f = bo.reshape(1, CQ).astype(np.float32)
    # bias [1,1,H,Q,K] -> biasT [kc, 128, H, Q]
    bT = bias[0, 0].transpose(2, 0, 1).reshape(2, CQ, H, S)
    bT = np.ascontiguousarray(bT).astype(BF16)

    in_maps = []
    for i in range(NCORES):
        n0 = i * NPER
        sl = slice(n0, n0 + NPER)
        xq = input_q[0, sl].transpose(0, 2, 1)      # [NPER, 128, 256]
        xk = input_k[0, sl].transpose(0, 2, 1)
        xv = input_v[0, sl].transpose(0, 2, 1)
        x_all = np.ascontiguousarray(
            np.stack([xq, xk, xv], axis=1)).astype(BF16)  # [NPER,3,128,256]
        m = mask[0, sl, 0, 0, :]                     # [NPER, 256]
        mT = np.ascontiguousarray(
            m.T.reshape(2, CQ, NPER).transpose(1, 0, 2).reshape(CQ, 2 * NPER)
        ).astype(np.float32)
        in_maps.append({
            "x_all": x_all, "maskT": mT, "biasT": bT,
            "wq": wqbd, "wk": wk_b, "wv": wv_b, "wg": wg_b,
            "wo_t": wo_tt, "bg": bg_b, "bo": bo_f,
        })

    res = run_bass_kernel_spmd(nc, in_maps, list(range(NCORES)))
    out = np.concatenate([r["out"][None] for r in res.results], axis=0)
    return out.reshape(1, N, S, CQ).astype(np.float32)


if __name__ == "__main__":
    rng = np.random.default_rng(0)
    inps = {
        "input_q": rng.standard_normal((B, N, S, CQ), dtype=np.float32),
        "input_k": rng.standard_normal((B, N, S, CQ), dtype=np.float32),
        "input_v": rng.standard_normal((B, N, S, CQ), dtype=np.float32),
        "mask": np.ones((B, N, 1, 1, S), dtype=np.float32),
        "bias": rng.standard_normal((B, 1, H, S, S), dtype=np.float32),
        "wq": rng.standard_normal((CQ, H * C), dtype=np.float32) * 0.05,
        "wk": rng.standard_normal((CQ, H * C), dtype=np.float32) * 0.05,
        "wv": rng.standard_normal((CQ, H * C), dtype=np.float32) * 0.05,
        "wg": rng.standard_normal((CQ, H * C), dtype=np.float32) * 0.05,
        "bg": np.ones((H * C,), dtype=np.float32),
        "wo": rng.standard_normal((H * C, CQ), dtype=np.float32) * 0.05,
        "bo": np.zeros((CQ,), dtype=np.float32),
    }
    out = kernel(**inps)
    print("out shape", out.shape, out.dtype, float(np.abs(out).mean()))


# revision 27
# speedup vs baseline: 1.1882x; 1.1882x over previous
"""Trainium2 Bass kernel for gated pair-bias attention (AlphaFold-style).

Reference computation per (b=1, n) row:
  q,k,v = proj(input_*) reshaped to [H=8, S=256, C=32]; q /= sqrt(32)
  a = softmax(q@k^T + (mask-1)*1e9 + bias)      # [H, Q, K]
  o = (a@v) * sigmoid(input_q@wg + bg)          # gated
  out = o @ wo + bo                             # [S, 128]

Sharding: dim 1 (N=256 rows) split across 8 cores, 32 rows/core.
Layout strategy (per row, all bf16 matmuls, fp32 PSUM):
  - host pre-transposes x to [c=128, s=256] (pure layout prep)
  - qT,kT computed transposed [d,s]; v,g computed natural [s,d]
  - logits computed transposed e^T[k,q] per head (K=32 row-group packed)
  - softmax without max-subtraction (logits are small); Z via an
    em-column appended to v in the AV matmul (em = exp((mask-1)*1e9))
  - exp(logits)*exp(bias^T) product form; bias^T preloaded once per core
  - AV natural: o[q, (h,33)] with Z in col 32 of each 33-block
  - normalize+gate on DVE, PE-transpose og -> og^T, final matmul natural
"""

import math
import sys

sys.path.insert(0, "/opt/trn_rl_repo")

import numpy as np
import ml_dtypes

BF16 = ml_dtypes.bfloat16

B, N, S, CQ = 1, 256, 256, 128
H, C = 8, 32
NCORES = 8
NPER = N // NCORES  # 32 rows per core


def _build_bass():
    import concourse.bass as bass
    import concourse.bacc as bacc
    import concourse.tile as tile
    from concourse import mybir
    from concourse.masks import make_identity

    dt = mybir.dt
    AF = mybir.ActivationFunctionType
    ALU = mybir.AluOpType

    nc = bacc.Bacc()

    # ---- DRAM parameters (per-core shapes) ----
    x_all = nc.declare_dram_parameter("x_all", [NPER, 3, CQ, S], dt.bfloat16, isOutput=False)
    maskT = nc.declare_dram_parameter("maskT", [CQ, 2 * NPER], dt.float32, isOutput=False)
    biasT = nc.declare_dram_parameter("biasT", [2, CQ, H, S], dt.bfloat16, isOutput=False)
    wq = nc.declare_dram_parameter("wq", [2, 4, CQ, CQ], dt.bfloat16, isOutput=False)
    wk = nc.declare_dram_parameter("wk", [CQ, H * C], dt.bfloat16, isOutput=False)
    wv = nc.declare_dram_parameter("wv", [CQ, H * C], dt.bfloat16, isOutput=False)
    wg = nc.declare_dram_parameter("wg", [CQ, H * C], dt.bfloat16, isOutput=False)
    wo_t = nc.declare_dram_parameter("wo_t", [CQ, 2 * CQ], dt.bfloat16, isOutput=False)
    bg = nc.declare_dram_parameter("bg", [1, H * C], dt.bfloat16, isOutput=False)
    bo = nc.declare_dram_parameter("bo", [1, CQ], dt.float32, isOutput=False)
    out_d = nc.declare_dram_parameter("out", [NPER, S, CQ], dt.float32, isOutput=True)

    with tile.TileContext(nc) as tc:
        with (
            tc.tile_pool(name="const", bufs=1) as const,
            tc.tile_pool(name="xp", bufs=4) as xp,
            tc.tile_pool(name="qk", bufs=2) as qkp,
            tc.tile_pool(name="ep", bufs=8) as ep,
            tc.tile_pool(name="eraw", bufs=3) as erawp,
            tc.tile_pool(name="vap", bufs=4) as vap,
            tc.tile_pool(name="gp", bufs=2) as gpool,
            tc.tile_pool(name="ogp", bufs=4) as ogp,
            tc.tile_pool(name="zp", bufs=4) as zp,
            tc.tile_pool(name="outp", bufs=2) as outp,
            tc.tile_pool(name="psbig", bufs=2, space="PSUM") as psbig,
            tc.tile_pool(name="psqkvg", bufs=2, space="PSUM") as psqkvg,
            tc.tile_pool(name="pspm", bufs=2, space="PSUM") as pspm,
        ):
            # ---------- once-per-core setup ----------
            wq_t = const.tile([CQ, 2, 4, CQ], dt.bfloat16, tag="wq")
            wk_t = const.tile([CQ, H * C], dt.bfloat16, tag="wk")
            wv_t = const.tile([CQ, H * C], dt.bfloat16, tag="wv")
            wg_t = const.tile([CQ, H * C], dt.bfloat16, tag="wg")
            wo_tt = const.tile([CQ, 2 * CQ], dt.bfloat16, tag="wo")
            bg_t = const.tile([1, H * C], dt.bfloat16, tag="bg")
            bo_t = const.tile([CQ, CQ], dt.float32, tag="bo")
            ones_c = const.tile([1, CQ], dt.bfloat16, tag="ones")
            ident = const.tile([CQ, CQ], dt.bfloat16, tag="ident")
            emf = const.tile([CQ, 2 * NPER], dt.float32, tag="emf")
            emb = const.tile([CQ, 2 * NPER], dt.bfloat16, tag="emb")
            ebr = const.tile([CQ, 2, H * S], dt.bfloat16, tag="ebr")
            eb = const.tile([CQ, 2, H * S], dt.bfloat16, tag="eb")
            mt_t = const.tile([CQ, 2 * NPER], dt.float32, tag="mt")

            nc.sync.dma_start(out=wq_t, in_=wq.ap().rearrange("a g p m -> p a g m"))
            nc.sync.dma_start(out=wk_t, in_=wk[:, :])
            nc.sync.dma_start(out=wv_t, in_=wv[:, :])
            nc.sync.dma_start(out=wg_t, in_=wg[:, :])
            nc.sync.dma_start(out=wo_tt, in_=wo_t[:, :])
            nc.sync.dma_start(out=bg_t, in_=bg[:, :])
            nc.sync.dma_start(out=mt_t, in_=maskT[:, :])
            # broadcast bo across 128 partitions
            bo_ap0 = bo[:, :]
            bo_bc_ap = bass.AP(tensor=bo_ap0.tensor, offset=bo_ap0.offset,
                               ap=[[0, CQ], [1, CQ]])
            nc.sync.dma_start(out=bo_t, in_=bo_bc_ap)
            for kc in range(2):
                nc.sync.dma_start(
                    out=ebr[:, kc],
                    in_=biasT[kc].rearrange("p h q -> p (h q)"),
                )
            nc.vector.memset(ones_c, 1.0)
            make_identity(nc, ident)
            # em = exp((mask-1)*1e9) as [k, (kc, n)] per-partition scalars
            nc.vector.tensor_scalar(out=emf, in0=mt_t, scalar1=1.0, scalar2=1.0e9,
                                    op0=ALU.subtract, op1=ALU.mult)
            nc.scalar.activation(emf, emf, AF.Exp)
            nc.vector.tensor_copy(out=emb, in_=emf)
            # expbiasT = exp(bias^T)  [128, kc, (h q)]
            for kc in range(2):
                nc.scalar.activation(eb[:, kc], ebr[:, kc], AF.Exp)

            # ---------- per-row pipeline ----------
            for n in range(NPER):
                # x^T tiles: [c=128, (t, s)] for t in (q, k, v)
                xt = xp.tile([CQ, 3, S], dt.bfloat16, tag="xt")
                nc.sync.dma_start(out=xt, in_=x_all[n].rearrange("t p s -> p t s"))
                xqT, xkT, xvT = xt[:, 0], xt[:, 1], xt[:, 2]

                # projections: k transposed [d, s]; q block-diagonal per head
                qbd = []
                for dc in range(2):
                    ps_qb = psbig.tile([CQ, 4, S], dt.float32, tag="big")
                    for g in range(4):
                        nc.tensor.matmul(ps_qb[:, g],
                                         wq_t[:, dc, g], xqT,
                                         start=True, stop=True)
                    qb = qkp.tile([CQ, 4, S], dt.bfloat16, tag="qbd")
                    nc.vector.tensor_copy(out=qb, in_=ps_qb)
                    qbd.append(qb)
                ps_k = psqkvg.tile([CQ, 2 * S], dt.float32, tag="psqkvg")
                for dc in range(2):
                    nc.tensor.matmul(ps_k[:, dc * S:(dc + 1) * S],
                                     wk_t[:, dc * CQ:(dc + 1) * CQ], xkT,
                                     start=True, stop=True)
                kT = qkp.tile([CQ, 2 * S], dt.bfloat16, tag="kT")
                nc.scalar.copy(out=kT, in_=ps_k)

                ps_v = psqkvg.tile([CQ, 2 * S], dt.float32, tag="psqkvg")
                ps_g = psqkvg.tile([CQ, 2 * S], dt.float32, tag="psqkvg")
                for sc in range(2):
                    nc.tensor.matmul(ps_v[:, sc * S:(sc + 1) * S],
                                     xvT[:, sc * CQ:(sc + 1) * CQ], wv_t,
                                     start=True, stop=True)
                for sc in range(2):
                    nc.tensor.matmul(ps_g[:, sc * S:(sc + 1) * S],
                                     xqT[:, sc * CQ:(sc + 1) * CQ], wg_t,
                                     start=True, stop=False)
                    nc.tensor.matmul(ps_g[:, sc * S:(sc + 1) * S],
                                     ones_c, bg_t, start=False, stop=True)

                # gate g = sigmoid(z) = 0.5*tanh(0.5 z)+0.5, natural [q,(sc,d)]
                g_t = gpool.tile([CQ, 2 * S], dt.bfloat16, tag="g")
                nc.scalar.activation(g_t, ps_g, AF.Tanh, scale=0.5)
                nc.vector.tensor_scalar(out=g_t, in0=g_t, scalar1=0.5, scalar2=0.5,
                                        op0=ALU.mult, op1=ALU.add)

                # v_aug [k-chunk, (h, 33)]: v*em | em
                vas = []
                for kc in range(2):
                    va = vap.tile([CQ, H, 33], dt.bfloat16, tag="va")
                    nc.scalar.activation(
                        va[:, :, 0:32],
                        ps_v[:, kc * S:(kc + 1) * S].rearrange("p (h x) -> p h x", x=32),
                        AF.Copy,
                        scale=emf[:, kc * NPER + n: kc * NPER + n + 1],
                    )
                    nc.vector.tensor_copy(
                        out=va[:, :, 32],
                        in_=emb[:, kc * NPER + n: kc * NPER + n + 1].broadcast_to((CQ, H)),
                    )
                    vas.append(va)

                # logits^T e[k, (g, q)] per (dc, kc): lhsT = full kT chunk,
                # rhs = block-diag q (zeros isolate each head) -> exp -> *expbias
                efin = [[None, None], [None, None]]
                for dc in range(2):
                    for kc in range(2):
                        ps_e = psbig.tile([CQ, 4, S], dt.float32, tag="big")
                        for gp in range(2):
                            nc.tensor.matmul(
                                ps_e[:, 2 * gp:2 * gp + 2, :],
                                kT[:, dc * S + kc * CQ: dc * S + kc * CQ + CQ],
                                qbd[dc][:, 2 * gp:2 * gp + 2, :],
                                start=True, stop=True,
                            )
                        er = erawp.tile([CQ, 4 * S], dt.bfloat16, tag="eraw")
                        nc.scalar.activation(er, ps_e.rearrange("p g s -> p (g s)"), AF.Exp)
                        ef = ep.tile([CQ, 4 * S], dt.bfloat16, tag="efin")
                        nc.vector.tensor_mul(
                            ef, er, eb[:, kc, dc * 4 * S:(dc + 1) * 4 * S])
                        efin[dc][kc] = ef

                # AV + normalize + gate; h/kc outer, qc inner (lhsT reuse)
                ogs = []
                ps_o0 = pspm.tile([CQ, H * 33], dt.float32, tag="pspm")
                ps_o1 = pspm.tile([CQ, H * 33], dt.float32, tag="pspm")
                ps_os = [ps_o0, ps_o1]
                for h in range(H):
                    hg, hh = h // 4, h % 4
                    for kc in range(2):
                        for qc in range(2):
                            nc.tensor.matmul(
                                ps_os[qc][:, h * 33:(h + 1) * 33],
                                efin[hg][kc][:, hh * S + qc * CQ: hh * S + qc * CQ + CQ],
                                vas[kc][:, h, :],
                                start=(kc == 0), stop=(kc == 1),
                            )
                for qc in range(2):
                    ps_o = ps_os[qc]
                    ps_o_r = ps_o.rearrange("p (h x) -> p h x", x=33)
                    zt = zp.tile([CQ, H], dt.float32, tag="zt")
                    rz = zp.tile([CQ, H], dt.float32, tag="rz")
                    nc.vector.tensor_copy(out=zt, in_=ps_o_r[:, :, 32])
                    nc.vector.reciprocal(out=rz, in_=zt)
                    og1 = ogp.tile([CQ, H, 32], dt.bfloat16, tag="og1")
                    nc.vector.tensor_mul(
                        og1, ps_o_r[:, :, 0:32],
                        rz.unsqueeze(2).broadcast_to((CQ, H, 32)))
                    og = ogp.tile([CQ, H * 32], dt.bfloat16, tag="og")
                    nc.vector.tensor_mul(
                        og, og1.rearrange("p h x -> p (h x)"),
                        g_t[:, qc * S:(qc + 1) * S])
                    ogs.append(og)

                # og^T via PE transpose, then final matmul (natural out)
                ogT = []
                for dc in range(2):
                    pt = pspm.tile([CQ, 2 * CQ], dt.bfloat16, tag="pspm")
                    for qc in range(2):
                        nc.tensor.transpose(
                            pt[:, qc * CQ:(qc + 1) * CQ],
                            ogs[qc][:, dc * CQ:(dc + 1) * CQ], ident)
                    ot = ogp.tile([CQ, 2 * CQ], dt.bfloat16, tag="ogT")
                    nc.scalar.copy(out=ot, in_=pt)
                    ogT.append(ot)

                ps_out = pspm.tile([CQ, 2 * CQ], dt.float32, tag="pspm")
                for sc in range(2):
                    for dc in range(2):
                        nc.tensor.matmul(
                            ps_out[:, sc * CQ:(sc + 1) * CQ],
                            ogT[dc][:, sc * CQ:(sc + 1) * CQ],
                            wo_tt[:, dc * CQ:(dc + 1) * CQ],
                            start=(dc == 0), stop=(dc == 1))
                out_sb = outp.tile([CQ, 2, CQ], dt.float32, tag="osb")
                nc.vector.tensor_add(
                    out_sb, ps_out.rearrange("p (sc c) -> p sc c", c=CQ),
                    bo_t.unsqueeze(1).broadcast_to((CQ, 2, CQ)))
                nc.sync.dma_start(
                    out=out_d[n].rearrange("(sc p) c -> p sc c", p=CQ),
                    in_=out_sb)
    if not nc.is_finalized():
        nc.finalize()
    return nc


_NC_CACHE = None


def _get_nc():
    global _NC_CACHE
    if _NC_CACHE is None:
        _NC_CACHE = _build_bass()
    return _NC_CACHE


_LDW_PATCHED = False


def _enable_ldw_opt():
    """Rewrite walrus argv to enable LDWEIGHTS elision for back-to-back
    matmuls sharing a stationary operand (correctness re-checked vs ref)."""
    global _LDW_PATCHED
    if _LDW_PATCHED:
        return
    from concourse import bass_utils as bu

    orig = bu.run_command

    def patched(argv, **kw):
        pass  # ldw-opt=true breaks walrus codegen (visitInstLdweights)
        return orig(argv, **kw)

    bu.run_command = patched
    _LDW_PATCHED = True


def kernel(input_q, input_k, input_v, mask, bias, wq, wk, wv, wg, bg, wo, bo):
    from concourse.bass_utils import run_bass_kernel_spmd

    _enable_ldw_opt()

    nc = _get_nc()

    # ---- host-side input prep (sharding + layout) ----
    wq_s = (wq / math.sqrt(C)).astype(np.float32)   # fold q-scaling into wq
    # block-diagonal zero-padded q weights: wqbd[dc, g, c, m] = wq_s[c, dc*128+m]
    # if m//32 == g else 0  -> per-head logits with base-0 matmul operands
    wqbd = np.zeros((2, 4, CQ, CQ), dtype=np.float32)
    for dc in range(2):
        for g in range(4):
            wqbd[dc, g, :, g * 32:(g + 1) * 32] = wq_s[:, dc * CQ + g * 32: dc * CQ + (g + 1) * 32]
    wqbd = wqbd.astype(BF16)
    wk_b, wv_b, wg_b = wk.astype(BF16), wv.astype(BF16), wg.astype(BF16)
    # wo [256,128] -> [128, (dc,128)]: wo_t[p, dc*128+c] = wo[dc*128+p, c]
    wo_tt = np.ascontiguousarray(
        wo.reshape(2, CQ, CQ).transpose(1, 0, 2).reshape(CQ, 2 * CQ)).astype(BF16)
    bg_b = bg.reshape(1, H * C).astype(BF16)
    bo_f = bo.reshape(1, CQ).astype(np.float32)
    # bias [1,1,H,Q,K] -> biasT [kc, 128, H, Q]
    bT = bias[0, 0].transpose(2, 0, 1).reshape(2, CQ, H, S)
    bT = np.ascontiguousarray(bT).astype(BF16)

    in_maps = []
    for i in range(NCORES):
        n0 = i * NPER
        sl = slice(n0, n0 + NPER)
        xq = input_q[0, sl].transpose(0, 2, 1)      # [NPER, 128, 256]
        xk = input_k[0, sl].transpose(0, 2, 1)
        xv = input_v[0, sl].transpose(0, 2, 1)
        x_all = np.ascontiguousarray(
            np.stack([xq, xk, xv], axis=1)).astype(BF16)  # [NPER,3,128,256]
        m = mask[0, sl, 0, 0, :]                     # [NPER, 256]
        mT = np.ascontiguousarray(
            m.T.reshape(2, CQ, NPER).transpose(1, 0, 2).reshape(CQ, 2 * NPER)
        ).astype(np.float32)
        in_maps.append({
            "x_all": x_all, "maskT": mT, "biasT": bT,
            "wq": wqbd, "wk": wk_b, "wv": wv_b, "wg": wg_b,
            "wo_t": wo_tt, "bg": bg_b, "bo": bo_f,
        })

    res = run_bass_kernel_spmd(nc, in_maps, list(range(NCORES)))
    out = np.concatenate([r["out"][None] for r in res.results], axis=0)
    return out.reshape(1, N, S, CQ).astype(np.float32)


if __name__ == "__main__":
    rng = np.random.default_rng(0)
    inps = {
        "input_q": rng.standard_normal((B, N, S, CQ), dtype=np.float32),
        "input_k": rng.standard_normal((B, N, S, CQ), dtype=np.float32),
        "input_v": rng.standard_normal((B, N, S, CQ), dtype=np.float32),
        "mask": np.ones((B, N, 1, 1, S), dtype=np.float32),
        "bias": rng.standard_normal((B, 1, H, S, S), dtype=np.float32),
        "wq": rng.standard_normal((CQ, H * C), dtype=np.float32) * 0.05,
        "wk": rng.standard_normal((CQ, H * C), dtype=np.float32) * 0.05,
        "wv": rng.standard_normal((CQ, H * C), dtype=np.float32) * 0.05,
        "wg": rng.standard_normal((CQ, H * C), dtype=np.float32) * 0.05,
        "bg": np.ones((H * C,), dtype=np.float32),
        "wo": rng.standard_normal((H * C, CQ), dtype=np.float32) * 0.05,
        "bo": np.zeros((CQ,), dtype=np.float32),
    }
    out = kernel(**inps)
    print("out shape", out.shape, out.dtype, float(np.abs(out).mean()))


# revision 28
# speedup vs baseline: 1.1978x; 1.0081x over previous
"""Trainium2 Bass kernel for gated pair-bias attention (AlphaFold-style).

Reference computation per (b=1, n) row:
  q,k,v = proj(input_*) reshaped to [H=8, S=256, C=32]; q /= sqrt(32)
  a = softmax(q@k^T + (mask-1)*1e9 + bias)      # [H, Q, K]
  o = (a@v) * sigmoid(input_q@wg + bg)          # gated
  out = o @ wo + bo                             # [S, 128]

Sharding: dim 1 (N=256 rows) split across 8 cores, 32 rows/core.
Layout strategy (per row, all bf16 matmuls, fp32 PSUM):
  - host pre-transposes x to [c=128, s=256] (pure layout prep)
  - qT,kT computed transposed [d,s]; v,g computed natural [s,d]
  - logits computed transposed e^T[k,q] per head (K=32 row-group packed)
  - softmax without max-subtraction (logits are small); Z via an
    em-column appended to v in the AV matmul (em = exp((mask-1)*1e9))
  - exp(logits)*exp(bias^T) product form; bias^T preloaded once per core
  - AV natural: o[q, (h,33)] with Z in col 32 of each 33-block
  - normalize+gate on DVE, PE-transpose og -> og^T, final matmul natural
"""

import math
import sys

sys.path.insert(0, "/opt/trn_rl_repo")

import numpy as np
import ml_dtypes

BF16 = ml_dtypes.bfloat16

B, N, S, CQ = 1, 256, 256, 128
H, C = 8, 32
NCORES = 8
NPER = N // NCORES  # 32 rows per core


def _build_bass():
    import concourse.bass as bass
    import concourse.bacc as bacc
    import concourse.tile as tile
    from concourse import mybir
    from concourse.masks import make_identity

    dt = mybir.dt
    AF = mybir.ActivationFunctionType
    ALU = mybir.AluOpType

    nc = bacc.Bacc()

    # ---- DRAM parameters (per-core shapes) ----
    x_all = nc.declare_dram_parameter("x_all", [NPER, 3, CQ, S], dt.bfloat16, isOutput=False)
    maskT = nc.declare_dram_parameter("maskT", [CQ, 2 * NPER], dt.float32, isOutput=False)
    biasT = nc.declare_dram_parameter("biasT", [2, CQ, H, S], dt.bfloat16, isOutput=False)
    wq = nc.declare_dram_parameter("wq", [2, 4, CQ, CQ], dt.bfloat16, isOutput=False)
    wk = nc.declare_dram_parameter("wk", [CQ, H * C], dt.bfloat16, isOutput=False)
    wv = nc.declare_dram_parameter("wv", [CQ, H * C], dt.bfloat16, isOutput=False)
    wg = nc.declare_dram_parameter("wg", [CQ, H * C], dt.bfloat16, isOutput=False)
    wo_t = nc.declare_dram_parameter("wo_t", [CQ, 2 * CQ], dt.bfloat16, isOutput=False)
    bg = nc.declare_dram_parameter("bg", [1, H * C], dt.bfloat16, isOutput=False)
    bo = nc.declare_dram_parameter("bo", [1, CQ], dt.float32, isOutput=False)
    out_d = nc.declare_dram_parameter("out", [NPER, S, CQ], dt.float32, isOutput=True)

    with tile.TileContext(nc) as tc:
        with (
            tc.tile_pool(name="const", bufs=1) as const,
            tc.tile_pool(name="xp", bufs=4) as xp,
            tc.tile_pool(name="qk", bufs=2) as qkp,
            tc.tile_pool(name="ep", bufs=8) as ep,
            tc.tile_pool(name="eraw", bufs=3) as erawp,
            tc.tile_pool(name="vap", bufs=4) as vap,
            tc.tile_pool(name="gp", bufs=2) as gpool,
            tc.tile_pool(name="ogp", bufs=4) as ogp,
            tc.tile_pool(name="zp", bufs=4) as zp,
            tc.tile_pool(name="outp", bufs=2) as outp,
            tc.tile_pool(name="psbig", bufs=2, space="PSUM") as psbig,
            tc.tile_pool(name="psqkvg", bufs=2, space="PSUM") as psqkvg,
            tc.tile_pool(name="pspm", bufs=2, space="PSUM") as pspm,
        ):
            # ---------- once-per-core setup ----------
            wq_t = const.tile([CQ, 2, 4, CQ], dt.bfloat16, tag="wq")
            wk_t = const.tile([CQ, H * C], dt.bfloat16, tag="wk")
            wv_t = const.tile([CQ, H * C], dt.bfloat16, tag="wv")
            wg_t = const.tile([CQ, H * C], dt.bfloat16, tag="wg")
            wo_tt = const.tile([CQ, 2 * CQ], dt.bfloat16, tag="wo")
            bg_t = const.tile([1, H * C], dt.bfloat16, tag="bg")
            bo_t = const.tile([CQ, CQ], dt.float32, tag="bo")
            ones_c = const.tile([1, CQ], dt.bfloat16, tag="ones")
            ident = const.tile([CQ, CQ], dt.bfloat16, tag="ident")
            emf = const.tile([CQ, 2 * NPER], dt.float32, tag="emf")
            emb = const.tile([CQ, 2 * NPER], dt.bfloat16, tag="emb")
            ebr = const.tile([CQ, 2, H * S], dt.bfloat16, tag="ebr")
            eb = const.tile([CQ, 2, H * S], dt.bfloat16, tag="eb")
            mt_t = const.tile([CQ, 2 * NPER], dt.float32, tag="mt")

            nc.sync.dma_start(out=wq_t, in_=wq.ap().rearrange("a g p m -> p a g m"))
            nc.sync.dma_start(out=wk_t, in_=wk[:, :])
            nc.sync.dma_start(out=wv_t, in_=wv[:, :])
            nc.sync.dma_start(out=wg_t, in_=wg[:, :])
            nc.sync.dma_start(out=wo_tt, in_=wo_t[:, :])
            nc.sync.dma_start(out=bg_t, in_=bg[:, :])
            nc.sync.dma_start(out=mt_t, in_=maskT[:, :])
            # broadcast bo across 128 partitions
            bo_ap0 = bo[:, :]
            bo_bc_ap = bass.AP(tensor=bo_ap0.tensor, offset=bo_ap0.offset,
                               ap=[[0, CQ], [1, CQ]])
            nc.sync.dma_start(out=bo_t, in_=bo_bc_ap)
            for kc in range(2):
                nc.sync.dma_start(
                    out=ebr[:, kc],
                    in_=biasT[kc].rearrange("p h q -> p (h q)"),
                )
            nc.vector.memset(ones_c, 1.0)
            make_identity(nc, ident)
            # em = exp((mask-1)*1e9) as [k, (kc, n)] per-partition scalars
            nc.vector.tensor_scalar(out=emf, in0=mt_t, scalar1=1.0, scalar2=1.0e9,
                                    op0=ALU.subtract, op1=ALU.mult)
            nc.scalar.activation(emf, emf, AF.Exp)
            nc.vector.tensor_copy(out=emb, in_=emf)
            # expbiasT = exp(bias^T)  [128, kc, (h q)]
            for kc in range(2):
                nc.scalar.activation(eb[:, kc], ebr[:, kc], AF.Exp)

            # ---------- per-row pipeline ----------
            for n in range(NPER):
                # x^T tiles: [c=128, (t, s)] for t in (q, k, v)
                xt = xp.tile([CQ, 3, S], dt.bfloat16, tag="xt")
                nc.sync.dma_start(out=xt, in_=x_all[n].rearrange("t p s -> p t s"))
                xqT, xkT, xvT = xt[:, 0], xt[:, 1], xt[:, 2]

                # projections: k transposed [d, s]; q block-diagonal per head
                qbd = []
                for dc in range(2):
                    ps_qb = psbig.tile([CQ, 4, S], dt.float32, tag="big")
                    for g in range(4):
                        nc.tensor.matmul(ps_qb[:, g],
                                         wq_t[:, dc, g], xqT,
                                         start=True, stop=True)
                    qb = qkp.tile([CQ, 4, S], dt.bfloat16, tag="qbd")
                    nc.vector.tensor_copy(out=qb, in_=ps_qb)
                    qbd.append(qb)
                ps_k = psqkvg.tile([CQ, 2 * S], dt.float32, tag="psqkvg")
                for dc in range(2):
                    nc.tensor.matmul(ps_k[:, dc * S:(dc + 1) * S],
                                     wk_t[:, dc * CQ:(dc + 1) * CQ], xkT,
                                     start=True, stop=True)
                kT = qkp.tile([CQ, 2 * S], dt.bfloat16, tag="kT")
                nc.scalar.copy(out=kT, in_=ps_k)

                ps_v = psqkvg.tile([CQ, 2 * S], dt.float32, tag="psqkvg")
                ps_g = psqkvg.tile([CQ, 2 * S], dt.float32, tag="psqkvg")
                for sc in range(2):
                    nc.tensor.matmul(ps_v[:, sc * S:(sc + 1) * S],
                                     xvT[:, sc * CQ:(sc + 1) * CQ], wv_t,
                                     start=True, stop=True)
                    nc.tensor.matmul(ps_g[:, sc * S:(sc + 1) * S],
                                     xqT[:, sc * CQ:(sc + 1) * CQ], wg_t,
                                     start=True, stop=False)
                    nc.tensor.matmul(ps_g[:, sc * S:(sc + 1) * S],
                                     ones_c, bg_t, start=False, stop=True)

                # gate g = sigmoid(z) = 0.5*tanh(0.5 z)+0.5, natural [q,(sc,d)]
                g_t = gpool.tile([CQ, 2 * S], dt.bfloat16, tag="g")
                nc.scalar.activation(g_t, ps_g, AF.Tanh, scale=0.5)
                nc.vector.tensor_scalar(out=g_t, in0=g_t, scalar1=0.5, scalar2=0.5,
                                        op0=ALU.mult, op1=ALU.add)

                # v_aug [k-chunk, (h, 33)]: v*em | em
                vas = []
                for kc in range(2):
                    va = vap.tile([CQ, H, 33], dt.bfloat16, tag="va")
                    nc.scalar.activation(
                        va[:, :, 0:32],
                        ps_v[:, kc * S:(kc + 1) * S].rearrange("p (h x) -> p h x", x=32),
                        AF.Copy,
                        scale=emf[:, kc * NPER + n: kc * NPER + n + 1],
                    )
                    nc.vector.tensor_copy(
                        out=va[:, :, 32],
                        in_=emb[:, kc * NPER + n: kc * NPER + n + 1].broadcast_to((CQ, H)),
                    )
                    vas.append(va)

                # logits^T e[k, (g, q)] per (dc, kc): lhsT = full kT chunk,
                # rhs = block-diag q (zeros isolate each head) -> exp -> *expbias
                efin = [[None, None], [None, None]]
                for dc in range(2):
                    for kc in range(2):
                        ps_e = psbig.tile([CQ, 4, S], dt.float32, tag="big")
                        for gp in range(2):
                            nc.tensor.matmul(
                                ps_e[:, 2 * gp:2 * gp + 2, :],
                                kT[:, dc * S + kc * CQ: dc * S + kc * CQ + CQ],
                                qbd[dc][:, 2 * gp:2 * gp + 2, :],
                                start=True, stop=True,
                            )
                        er = erawp.tile([CQ, 4 * S], dt.bfloat16, tag="eraw")
                        nc.scalar.activation(er, ps_e.rearrange("p g s -> p (g s)"), AF.Exp)
                        ef = ep.tile([CQ, 4 * S], dt.bfloat16, tag="efin")
                        nc.vector.tensor_mul(
                            ef, er, eb[:, kc, dc * 4 * S:(dc + 1) * 4 * S])
                        efin[dc][kc] = ef

                # AV + normalize + gate; h/kc outer, qc inner (lhsT reuse)
                ogs = []
                ps_o0 = pspm.tile([CQ, H * 33], dt.float32, tag="pspm")
                ps_o1 = pspm.tile([CQ, H * 33], dt.float32, tag="pspm")
                ps_os = [ps_o0, ps_o1]
                for h in range(H):
                    hg, hh = h // 4, h % 4
                    for kc in range(2):
                        for qc in range(2):
                            nc.tensor.matmul(
                                ps_os[qc][:, h * 33:(h + 1) * 33],
                                efin[hg][kc][:, hh * S + qc * CQ: hh * S + qc * CQ + CQ],
                                vas[kc][:, h, :],
                                start=(kc == 0), stop=(kc == 1),
                            )
                for qc in range(2):
                    ps_o = ps_os[qc]
                    ps_o_r = ps_o.rearrange("p (h x) -> p h x", x=33)
                    zt = zp.tile([CQ, H], dt.float32, tag="zt")
                    rz = zp.tile([CQ, H], dt.float32, tag="rz")
                    nc.vector.tensor_copy(out=zt, in_=ps_o_r[:, :, 32])
                    nc.vector.reciprocal(out=rz, in_=zt)
                    og1 = ogp.tile([CQ, H, 32], dt.bfloat16, tag="og1")
                    nc.vector.tensor_mul(
                        og1, ps_o_r[:, :, 0:32],
                        rz.unsqueeze(2).broadcast_to((CQ, H, 32)))
                    og = ogp.tile([CQ, H * 32], dt.bfloat16, tag="og")
                    nc.vector.tensor_mul(
                        og, og1.rearrange("p h x -> p (h x)"),
                        g_t[:, qc * S:(qc + 1) * S])
                    ogs.append(og)

                # og^T via PE transpose, then final matmul (natural out)
                ogT = []
                for dc in range(2):
                    pt = pspm.tile([CQ, 2 * CQ], dt.bfloat16, tag="pspm")
                    for qc in range(2):
                        nc.tensor.transpose(
                            pt[:, qc * CQ:(qc + 1) * CQ],
                            ogs[qc][:, dc * CQ:(dc + 1) * CQ], ident)
                    ot = ogp.tile([CQ, 2 * CQ], dt.bfloat16, tag="ogT")
                    nc.scalar.copy(out=ot, in_=pt)
                    ogT.append(ot)

                ps_out = pspm.tile([CQ, 2 * CQ], dt.float32, tag="pspm")
                for sc in range(2):
                    for dc in range(2):
                        nc.tensor.matmul(
                            ps_out[:, sc * CQ:(sc + 1) * CQ],
                            ogT[dc][:, sc * CQ:(sc + 1) * CQ],
                            wo_tt[:, dc * CQ:(dc + 1) * CQ],
                            start=(dc == 0), stop=(dc == 1))
                out_sb = outp.tile([CQ, 2, CQ], dt.float32, tag="osb")
                nc.vector.tensor_add(
                    out_sb, ps_out.rearrange("p (sc c) -> p sc c", c=CQ),
                    bo_t.unsqueeze(1).broadcast_to((CQ, 2, CQ)))
                nc.sync.dma_start(
                    out=out_d[n].rearrange("(sc p) c -> p sc c", p=CQ),
                    in_=out_sb)
    if not nc.is_finalized():
        nc.finalize()
    return nc


_NC_CACHE = None


def _get_nc():
    global _NC_CACHE
    if _NC_CACHE is None:
        _NC_CACHE = _build_bass()
    return _NC_CACHE


_LDW_PATCHED = False


def _enable_ldw_opt():
    """Rewrite walrus argv to enable LDWEIGHTS elision for back-to-back
    matmuls sharing a stationary operand (correctness re-checked vs ref)."""
    global _LDW_PATCHED
    if _LDW_PATCHED:
        return
    from concourse import bass_utils as bu

    orig = bu.run_command

    def patched(argv, **kw):
        pass  # ldw-opt=true breaks walrus codegen (visitInstLdweights)
        return orig(argv, **kw)

    bu.run_command = patched
    _LDW_PATCHED = True


def kernel(input_q, input_k, input_v, mask, bias, wq, wk, wv, wg, bg, wo, bo):
    from concourse.bass_utils import run_bass_kernel_spmd

    _enable_ldw_opt()

    nc = _get_nc()

    # ---- host-side input prep (sharding + layout) ----
    wq_s = (wq / math.sqrt(C)).astype(np.float32)   # fold q-scaling into wq
    # block-diagonal zero-padded q weights: wqbd[dc, g, c, m] = wq_s[c, dc*128+m]
    # if m//32 == g else 0  -> per-head logits with base-0 matmul operands
    wqbd = np.zeros((2, 4, CQ, CQ), dtype=np.float32)
    for dc in range(2):
        for g in range(4):
            wqbd[dc, g, :, g * 32:(g + 1) * 32] = wq_s[:, dc * CQ + g * 32: dc * CQ + (g + 1) * 32]
    wqbd = wqbd.astype(BF16)
    wk_b, wv_b, wg_b = wk.astype(BF16), wv.astype(BF16), wg.astype(BF16)
    # wo [256,128] -> [128, (dc,128)]: wo_t[p, dc*128+c] = wo[dc*128+p, c]
    wo_tt = np.ascontiguousarray(
        wo.reshape(2, CQ, CQ).transpose(1, 0, 2).reshape(CQ, 2 * CQ)).astype(BF16)
    bg_b = bg.reshape(1, H * C).astype(BF16)
    bo_f = bo.reshape(1, CQ).astype(np.float32)
    # bias [1,1,H,Q,K] -> biasT [kc, 128, H, Q]
    bT = bias[0, 0].transpose(2, 0, 1).reshape(2, CQ, H, S)
    bT = np.ascontiguousarray(bT).astype(BF16)

    in_maps = []
    for i in range(NCORES):
        n0 = i * NPER
        sl = slice(n0, n0 + NPER)
        xq = input_q[0, sl].transpose(0, 2, 1)      # [NPER, 128, 256]
        xk = input_k[0, sl].transpose(0, 2, 1)
        xv = input_v[0, sl].transpose(0, 2, 1)
        x_all = np.ascontiguousarray(
            np.stack([xq, xk, xv], axis=1)).astype(BF16)  # [NPER,3,128,256]
        m = mask[0, sl, 0, 0, :]                     # [NPER, 256]
        mT = np.ascontiguousarray(
            m.T.reshape(2, CQ, NPER).transpose(1, 0, 2).reshape(CQ, 2 * NPER)
        ).astype(np.float32)
        in_maps.append({
            "x_all": x_all, "maskT": mT, "biasT": bT,
            "wq": wqbd, "wk": wk_b, "wv": wv_b, "wg": wg_b,
            "wo_t": wo_tt, "bg": bg_b, "bo": bo_f,
        })

    res = run_bass_kernel_spmd(nc, in_maps, list(range(NCORES)))
    out = np.concatenate([r["out"][None] for r in res.results], axis=0)
    return out.reshape(1, N, S, CQ).astype(np.float32)


if __name__ == "__main__":
    rng = np.random.default_rng(0)
    inps = {
        "input_q": rng.standard_normal((B, N, S, CQ), dtype=np.float32),
        "input_k": rng.standard_normal((B, N, S, CQ), dtype=np.float32),
        "input_v": rng.standard_normal((B, N, S, CQ), dtype=np.float32),
        "mask": np.ones((B, N, 1, 1, S), dtype=np.float32),
        "bias": rng.standard_normal((B, 1, H, S, S), dtype=np.float32),
        "wq": rng.standard_normal((CQ, H * C), dtype=np.float32) * 0.05,
        "wk": rng.standard_normal((CQ, H * C), dtype=np.float32) * 0.05,
        "wv": rng.standard_normal((CQ, H * C), dtype=np.float32) * 0.05,
        "wg": rng.standard_normal((CQ, H * C), dtype=np.float32) * 0.05,
        "bg": np.ones((H * C,), dtype=np.float32),
        "wo": rng.standard_normal((H * C, CQ), dtype=np.float32) * 0.05,
        "bo": np.zeros((CQ,), dtype=np.float32),
    }
    out = kernel(**inps)
    print("out shape", out.shape, out.dtype, float(np.abs(out).mean()))
